# revision 32
# baseline (speedup 1.0000x reference)
"""Trainium2 Bass kernel for nn_BDH_4406636445711 (dense transformer).

Sharding: 8 cores = data-parallel over B(2) x tensor-parallel over H(4).
Core c handles (b = c//4, h = c%4): its head's Dx/Dy slices, E rows, and a
V/4 shard of the readout. Per layer the y@E partial is AllReduced within
each b-group of 4 cores; v stays replicated inside the group. The host
stitches the 8 per-core [VS, T] logit shards into the full [B, T, V].

v4 design notes:
- Quarter-T (512) granularity C->ln->D pipeline; PSUM = pmm512(2) +
  psacc(4) + pstat(2) = 8 banks.
- bf16 everywhere except the f32r residual vT, LN stat chains, and pos.
- x lives in SBUF (xsb, 32KB/part) - no DRAM spill.
- ln chains: colsum matmuls -> ACT mul/Square -> DVE var ->
  ACT Sqrt -> DVE reciprocal_approx_fast -> GPSIMD partition_broadcast.
- ln(w) mean shortcut: sum_d w = sum_d pos (v, ln(u) are LN outputs).
- Aggressive overlap: E(th0)/phaseA(th0)/B_x(th0) emitted mid-layer
  (after D(q2)) under C-phase PE work; vp_td double-buffered per layer so
  next-layer transposes don't WAR current a-accum; RoPE(th0) overlaps
  E(th1)'s AllReduce wait; next-layer C(q0) kc-loop walks RoPE pair order
  so scores chase RoPE output. Last layer: readout(th0) fills AR(th1).
"""

import os
import sys

sys.path.insert(0, "/opt/trn_rl_repo")

import numpy as np

import concourse.bass as bass
import concourse.tile as tile
from concourse import bacc, mybir
from concourse.bass_utils import run_bass_kernel_spmd
from concourse.masks import make_identity
from concourse import library_config

F32 = mybir.dt.float32
F32R = mybir.dt.float32r
BF16 = mybir.dt.bfloat16
I32 = mybir.dt.int32
AF = mybir.ActivationFunctionType
OP = mybir.AluOpType

B, T, H, D, K, V, L = 2, 2048, 4, 256, 1024, 32000, 6
VS = V // 4
EPS = 1e-5
NT = T // 128
NKT = K // 128
ND = D // 128
TH = T // 2
TQ = 512
NQ = T // TQ
PAIR_KC = [0, 4, 1, 5, 2, 6, 3, 7]   # kc order matching RoPE pair emission

N_LAYERS = int(os.environ.get("KRN_LAYERS", str(L)))
DO_READOUT = os.environ.get("KRN_READOUT", "1") == "1"


def build(nc):
    tok_d = nc.dram_tensor("tok", [T], I32, kind="ExternalInput")
    emb_d = nc.dram_tensor("emb", [V, D], F32, kind="ExternalInput")
    posT_d = nc.dram_tensor("posT", [D, T], BF16, kind="ExternalInput")
    dx_d = nc.dram_tensor("dx", [D, K], BF16, kind="ExternalInput")
    dy_d = nc.dram_tensor("dy", [D, K], BF16, kind="ExternalInput")
    e_d = nc.dram_tensor("eh", [K, D], BF16, kind="ExternalInput")
    ro_d = nc.dram_tensor("ro", [D, VS], BF16, kind="ExternalInput")
    cos_d = nc.dram_tensor("cosh", [4, 128, T], BF16, kind="ExternalInput")
    sin_d = nc.dram_tensor("sinh", [4, 128, T], BF16, kind="ExternalInput")
    nmw_d = nc.dram_tensor("nmw", [128, T], BF16, kind="ExternalInput")
    mw2_d = nc.dram_tensor("mw2", [1, T], BF16, kind="ExternalInput")
    out_d = nc.dram_tensor("logitsT", [VS, T], BF16, kind="ExternalOutput")

    groups = [[0, 1, 2, 3], [4, 5, 6, 7]]

    from contextlib import ExitStack

    with tile.TileContext(nc) as tc, ExitStack() as es:
        ent = es.enter_context
        ent(nc.allow_low_precision(reason="bf16/f32r rounding is intentional"))
        pp = ent(tc.tile_pool(name="persist", bufs=1))
        csp = ent(tc.tile_pool(name="cs", bufs=1))       # [128,8,TH] bf16 cos+sin
        rpp = ent(tc.tile_pool(name="rope", bufs=2))     # [128,TH] bf16
        scp = ent(tc.tile_pool(name="scr", bufs=2))      # [128,TQ] bf16
        atp = ent(tc.tile_pool(name="at", bufs=1))       # [128,ND,TQ] f32r
        sqp = ent(tc.tile_pool(name="sq", bufs=1))       # [128,ND,TQ] f32r
        lnp = ent(tc.tile_pool(name="ln", bufs=2))       # [128,ND,TQ] bf16
        ytp = ent(tc.tile_pool(name="yt", bufs=2))       # [128,TQ] bf16
        yep = ent(tc.tile_pool(name="ye", bufs=1))       # [128,ND,TQ] bf16
        utp = ent(tc.tile_pool(name="ut", bufs=1))       # [128,ND,TH] bf16
        squp = ent(tc.tile_pool(name="squ", bufs=1))     # [128,ND,TQ] f32r
        smp = ent(tc.tile_pool(name="sm", bufs=4))       # [1,TQ] f32 smalls
        bcp = ent(tc.tile_pool(name="bc", bufs=2))       # [128,TH] f32 bcasts
        tmpp = ent(tc.tile_pool(name="tmp", bufs=2))     # [128,TH] f32 temps
        posp = ent(tc.tile_pool(name="pos", bufs=1))     # [128,ND,TH] bf16
        gatp = ent(tc.tile_pool(name="gat", bufs=2))     # embed tiles
        lop = ent(tc.tile_pool(name="lo", bufs=1))       # [128,2,TH] bf16
        rop = ent(tc.tile_pool(name="rob", bufs=1))      # [128,ND,1024] bf16
        psm = ent(tc.tile_pool(name="pmm512", bufs=2, space="PSUM"))
        psa = ent(tc.tile_pool(name="psacc", bufs=2, space="PSUM"))
        pst = ent(tc.tile_pool(name="pstat", bufs=2, space="PSUM"))
        dpool = ent(tc.tile_pool(name="dram", bufs=1, space="DRAM"))
        if True:
            _ctr = [0]

            def _nm(p):
                _ctr[0] += 1
                return f"{p}{_ctr[0]}"

            # ---- constants ----
            ident_f = smp.tile([128, 128], F32, tag="sm", name="identf")
            make_identity(nc, ident_f[:])
            ident_r = pp.tile([128, 128], F32R)
            nc.vector.tensor_copy(ident_r[:], ident_f[:])
            ones_pf = pp.tile([128, 1], F32)
            nc.vector.memset(ones_pf[:], 1.0)
            ones_p = pp.tile([128, 1], F32R)
            nc.vector.tensor_copy(ones_p[:], ones_pf[:])
            ones_b = pp.tile([128, 1], BF16)
            nc.vector.tensor_copy(ones_b[:], ones_pf[:])
            eps_p = pp.tile([128, 1], F32)
            nc.vector.memset(eps_p[:], EPS)
            eps_1 = pp.tile([1, 1], F32)
            nc.vector.memset(eps_1[:], EPS)
            nc.gpsimd.load_library(library_config.attn)

            # ---- persistent tensors ----
            vT = pp.tile([128, ND, T], F32R)       # residual master (dT layout)
            vTb = pp.tile([128, ND, T], BF16)      # bf16 shadow
            qT = pp.tile([128, NKT, T], BF16)      # RoPE'd q (kT layout)
            xsb = pp.tile([128, NKT, T], BF16)     # x = relu(v@Dx) (kT layout)
            vp_td = [pp.tile([128, NT, D], BF16, name=f"vp{i}") for i in range(2)]
            dx_sb = pp.tile([128, ND, K], BF16)
            nc.sync.dma_start(dx_sb[:], dx_d.ap().rearrange("(c p) k -> p c k", p=128))
            dy_sb = pp.tile([128, ND, K], BF16)
            nc.sync.dma_start(dy_sb[:], dy_d.ap().rearrange("(c p) k -> p c k", p=128))
            e_sb = pp.tile([128, NKT, D], BF16)
            nc.sync.dma_start(e_sb[:], e_d.ap().rearrange("(c p) d -> p c d", p=128))
            nmw_sb = pp.tile([128, T], BF16)
            nc.sync.dma_start(nmw_sb[:], nmw_d.ap())
            mw2_sb = pp.tile([1, T], BF16)
            nc.sync.dma_start(mw2_sb[:], mw2_d.ap())

            cc_in = [dpool.tile([ND, 128, TH], BF16, tag=f"cci{i}", name=f"cci{i}")
                     for i in range(2)]
            cc_out = [dpool.tile([ND, 128, TH], BF16, tag=f"cco{i}", name=f"cco{i}")
                      for i in range(2)]

            def ln_chain(s1, s2, want_nm, mw2_ap, nm_dst, rs_dst):
                """LN stats from [1,TQ] psum colsums into bcast tile slices."""
                if want_nm:
                    nm_sb = smp.tile([1, TQ], F32, tag="sm", name=_nm("nm_"))
                    nc.scalar.mul(nm_sb[:], s1, -1.0 / D)
                    m2 = smp.tile([1, TQ], F32, tag="sm", name=_nm("m2_"))
                    nc.scalar.activation(m2[:], nm_sb[:], AF.Square)
                    nc.gpsimd.partition_broadcast(nm_dst, nm_sb[:])
                    m2_ap = m2[:]
                else:
                    m2_ap = mw2_ap
                var = smp.tile([1, TQ], F32, tag="sm", name=_nm("var_"))
                nc.vector.scalar_tensor_tensor(
                    out=var[:], in0=s2, scalar=1.0 / D, in1=m2_ap,
                    op0=OP.mult, op1=OP.subtract)
                std = smp.tile([1, TQ], F32, tag="sm", name=_nm("std_"))
                nc.scalar.activation(std[:], var[:], AF.Sqrt, bias=eps_1[:])
                rs = smp.tile([1, TQ], F32, tag="sm", name=_nm("rs_"))
                nc.vector.reciprocal_approx_fast(rs[:], std[:])
                nc.gpsimd.partition_broadcast(rs_dst, rs[:])

            def colsums(dst, src_bf=None, src_f=None):
                """dst [1,TQ] psum += ones^T @ src over ND blocks."""
                for dc in range(ND):
                    if src_bf is not None:
                        nc.tensor.matmul(dst, ones_b[:], src_bf(dc),
                                         start=(dc == 0), stop=(dc == ND - 1),
                                         skip_group_check=True)
                    else:
                        nc.tensor.matmul(dst, ones_p[:], src_f(dc),
                                         start=(dc == 0), stop=(dc == ND - 1),
                                         skip_group_check=True)

            def phaseA(th, layer):
                """vT[:, th] += pos; vTb cast; transpose into vp_td[next]."""
                t0 = th * TH
                vp = vp_td[(layer + 1) % 2]
                for dc in range(ND):
                    pch = posp.tile([128, TH], BF16, tag="pos", name=_nm("pos_"))
                    nc.sync.dma_start(
                        pch[:], posT_d.ap()[dc * 128:(dc + 1) * 128,
                                            t0:t0 + TH])
                    nc.vector.tensor_add(
                        vT[:, dc, t0:t0 + TH],
                        vT[:, dc, t0:t0 + TH].bitcast(F32), pch[:])
                    nc.vector.tensor_copy(vTb[:, dc, t0:t0 + TH],
                                          vT[:, dc, t0:t0 + TH])
                for dc in range(ND):
                    for g in range(2):
                        tp = psm.tile([128, 4, 128], F32R, tag="mm", name=_nm("tp_"))
                        for k in range(4):
                            n = th * 8 + g * 4 + k
                            nc.tensor.transpose(
                                out=tp[:, k, :],
                                in_=vT[:, dc, n * 128:(n + 1) * 128],
                                identity=ident_r[:])
                        nc.vector.tensor_copy(
                            vp[:, th * 8 + g * 4:th * 8 + (g + 1) * 4,
                               dc * 128:(dc + 1) * 128], tp[:])

            def phaseBx(th):
                """x = relu(v @ Dx) for this t-half into xsb (ACT writes)."""
                t0 = th * TH
                for ii in range(NKT):
                    for qq in range(2):
                        px = psm.tile([128, TQ], F32, tag="mm", name=_nm("px_"))
                        for dc in range(ND):
                            nc.tensor.matmul(
                                px[:],
                                dx_sb[:, dc, ii * 128:(ii + 1) * 128],
                                vTb[:, dc, t0 + qq * TQ:t0 + (qq + 1) * TQ],
                                start=(dc == 0), stop=(dc == ND - 1))
                        nc.scalar.activation(
                            xsb[:, ii, t0 + qq * TQ:t0 + (qq + 1) * TQ],
                            px[:], AF.Relu)

            def phaseRope(th):
                """qT = RoPE(x) for this t-half (DVE, bf16, pair order)."""
                t0 = th * TH
                gsl = slice(t0, t0 + TH)
                for i in range(4):
                    if i % 2 == 0:
                        cs = csp.tile([128, 4, TH], BF16, tag="cs",
                                      name=_nm("cs_"))
                        nc.scalar.dma_start(
                            cs[:, 0:2, :],
                            cos_d.ap()[i:i + 2, :, t0:t0 + TH].rearrange(
                                "c p t -> p c t"))
                        nc.scalar.dma_start(
                            cs[:, 2:4, :],
                            sin_d.ap()[i:i + 2, :, t0:t0 + TH].rearrange(
                                "c p t -> p c t"))
                    xi = xsb[:, i, gsl]
                    xj = xsb[:, i + 4, gsl]
                    cos_t = cs[:, i % 2, :]
                    sin_t = cs[:, 2 + i % 2, :]
                    m1 = rpp.tile([128, TH], BF16, tag="rp", name=_nm("m1_"))
                    nc.vector.tensor_mul(m1[:], xi, cos_t)
                    m2 = rpp.tile([128, TH], BF16, tag="rp", name=_nm("m2_"))
                    nc.vector.tensor_mul(m2[:], xj, sin_t)
                    nc.vector.tensor_sub(qT[:, i, gsl], m1[:], m2[:])
                    m3 = rpp.tile([128, TH], BF16, tag="rp", name=_nm("m3_"))
                    nc.vector.tensor_mul(m3[:], xj, cos_t)
                    m4 = rpp.tile([128, TH], BF16, tag="rp", name=_nm("m4_"))
                    nc.vector.tensor_mul(m4[:], xi, sin_t)
                    nc.vector.tensor_add(qT[:, i + 4, gsl], m3[:], m4[:])

            def phaseC(q, layer):
                """scores for quarter q; a-accum; ln(a) -> lnA (bf16)."""
                q0 = q * TQ
                vp = vp_td[layer % 2]
                pa = psa.tile([128, ND, TQ], F32, tag="acc", name=_nm("pa_"))
                for j in range(NT):
                    pscr = psm.tile([128, TQ], F32, tag="mm", name=_nm("ps_"))
                    for ki, kc in enumerate(PAIR_KC):
                        nc.tensor.matmul(
                            pscr[:],
                            qT[:, kc, j * 128:(j + 1) * 128],
                            qT[:, kc, q0:q0 + TQ],
                            start=(ki == 0), stop=(ki == NKT - 1))
                    scr = scp.tile([128, TQ], BF16, tag="scr", name=_nm("sc_"))
                    nc.scalar.copy(scr[:], pscr[:])
                    for dc in range(ND):
                        nc.tensor.matmul(
                            pa[:, dc, :],
                            vp[:, j, dc * 128:(dc + 1) * 128],
                            scr[:],
                            start=(j == 0), stop=(j == NT - 1),
                            skip_group_check=True)
                aT = atp.tile([128, ND, TQ], BF16, tag="at", name=_nm("aT_"))
                nc.vector.tensor_copy(aT[:], pa[:])
                sq = sqp.tile([128, ND, TQ], BF16, tag="sq", name=_nm("sq_"))
                nc.scalar.activation(sq[:], pa[:].bitcast(F32), AF.Square)
                s1t = pst.tile([1, TQ], F32, tag="st", name=_nm("s1_"))
                colsums(s1t[:], src_bf=lambda dc: aT[:, dc, :])
                s2t = pst.tile([1, TQ], F32, tag="st", name=_nm("s2_"))
                colsums(s2t[:], src_bf=lambda dc: sq[:, dc, :])
                nm_b = bcp.tile([128, TQ], F32, tag="bc", name=_nm("nmb_"))
                rs_b = bcp.tile([128, TQ], F32, tag="bc", name=_nm("rsb_"))
                ln_chain(s1t[:], s2t[:], True, None, nm_b[:], rs_b[:])
                lnA = lnp.tile([128, ND, TQ], BF16, tag="ln", name=_nm("lnA_"))
                for dc in range(ND):
                    cent = ytp.tile([128, TQ], BF16, tag="yt", name=_nm("ce_"))
                    nc.vector.tensor_add(cent[:], aT[:, dc, :], nm_b[:])
                    nc.vector.tensor_mul(lnA[:, dc, :], cent[:], rs_b[:])
                return lnA

            def phaseD(q, lnA):
                """y_i = relu(lnA @ Dy_i) * x_i; yE accum; ship to cc_in."""
                q0 = q * TQ
                pye = psa.tile([128, ND, TQ], F32, tag="acc", name=_nm("pye_"))
                for i in range(NKT):
                    py = psm.tile([128, TQ], F32, tag="mm", name=_nm("py_"))
                    for dc in range(ND):
                        nc.tensor.matmul(
                            py[:],
                            dy_sb[:, dc, i * 128:(i + 1) * 128],
                            lnA[:, dc, :],
                            start=(dc == 0), stop=(dc == ND - 1))
                    yt = ytp.tile([128, TQ], BF16, tag="yt", name=_nm("yt_"))
                    nc.vector.scalar_tensor_tensor(
                        out=yt[:], in0=py[:], scalar=0.0,
                        in1=xsb[:, i, q0:q0 + TQ],
                        op0=OP.max, op1=OP.mult)
                    for dc in range(ND):
                        nc.tensor.matmul(
                            pye[:, dc, :],
                            e_sb[:, i, dc * 128:(dc + 1) * 128],
                            yt[:],
                            start=(i == 0), stop=(i == NKT - 1),
                            skip_group_check=True)
                ye = yep.tile([128, ND, TQ], BF16, tag="ye", name=_nm("ye_"))
                nc.vector.tensor_copy(ye[:], pye[:])
                th, qq = q // 2, q % 2
                nc.sync.dma_start(
                    cc_in[th][:][:, :, qq * TQ:(qq + 1) * TQ].rearrange(
                        "c p t -> p c t"), ye[:])

            def phaseE(th):
                """u = ln(AR sum); w = vp + u; v = ln(w)."""
                t0 = th * TH
                uT = utp.tile([128, ND, TH], BF16, tag="ut", name=_nm("uT_"))
                nc.sync.dma_start(
                    uT[:], cc_out[th][:].rearrange("c p t -> p c t"))
                nmu_b = bcp.tile([128, TH], F32, tag="bcth", name=_nm("nmu_"))
                rsu_b = bcp.tile([128, TH], F32, tag="bcth", name=_nm("rsu_"))
                rsw_b = bcp.tile([128, TH], F32, tag="bcth", name=_nm("rsw_"))
                for qq in range(2):
                    sl = slice(qq * TQ, (qq + 1) * TQ)
                    squ = squp.tile([128, ND, TQ], BF16, tag="squ", name=_nm("su_"))
                    nc.scalar.activation(squ[:], uT[:, :, sl], AF.Square)
                    s1u = pst.tile([1, TQ], F32, tag="st", name=_nm("s1u_"))
                    colsums(s1u[:], src_bf=lambda dc: uT[:, dc, sl])
                    s2u = pst.tile([1, TQ], F32, tag="st", name=_nm("s2u_"))
                    colsums(s2u[:], src_bf=lambda dc: squ[:, dc, :])
                    ln_chain(s1u[:], s2u[:], True, None,
                             nmu_b[:, sl], rsu_b[:, sl])
                for dc in range(ND):
                    gsl = slice(t0, t0 + TH)
                    cent = tmpp.tile([128, TH], F32, tag="tmpth", name=_nm("cu_"))
                    nc.vector.tensor_add(cent[:], uT[:, dc, :], nmu_b[:])
                    lnu = tmpp.tile([128, TH], F32, tag="tmpth", name=_nm("lu_"))
                    nc.vector.tensor_mul(lnu[:], cent[:], rsu_b[:])
                    nc.vector.tensor_add(vT[:, dc, gsl],
                                         vT[:, dc, gsl].bitcast(F32), lnu[:])
                # ln(w): mean precomputed from pos colsums
                for qq in range(2):
                    sl = slice(qq * TQ, (qq + 1) * TQ)
                    gsl = slice(t0 + qq * TQ, t0 + (qq + 1) * TQ)
                    sqw = squp.tile([128, ND, TQ], BF16, tag="squ", name=_nm("sw_"))
                    nc.scalar.activation(sqw[:], vT[:, :, gsl].bitcast(F32),
                                         AF.Square)
                    s2w = pst.tile([1, TQ], F32, tag="st", name=_nm("s2w_"))
                    colsums(s2w[:], src_bf=lambda dc: sqw[:, dc, :])
                    ln_chain(None, s2w[:], False,
                             mw2_sb[:, t0 + qq * TQ:t0 + (qq + 1) * TQ],
                             None, rsw_b[:, sl])
                for dc in range(ND):
                    gsl = slice(t0, t0 + TH)
                    cent = tmpp.tile([128, TH], F32, tag="tmpth", name=_nm("cw_"))
                    nc.vector.tensor_add(cent[:], vT[:, dc, gsl].bitcast(F32),
                                         nmw_sb[:, gsl])
                    nc.vector.tensor_mul(vT[:, dc, gsl], cent[:], rsw_b[:])

            def fire_ar(th):
                nc.gpsimd.collective_compute(
                    "AllReduce", OP.add, replica_groups=groups,
                    ins=[cc_in[th][:].opt()], outs=[cc_out[th][:].opt()])

            def cast_vTb(th):
                t0 = th * TH
                for dc in range(ND):
                    nc.vector.tensor_copy(vTb[:, dc, t0:t0 + TH],
                                          vT[:, dc, t0:t0 + TH])

            def readout_half(th):
                t0 = th * TH
                nvg = 8  # groups of 8 vocab blocks (last ragged)
                for vg in range(nvg):
                    vbase = vg * 1024
                    gw = min(1024, VS - vbase)
                    ro_sb = rop.tile([128, ND, 1024], BF16, tag="ro",
                                     name=_nm("ro_"))
                    nc.sync.dma_start(
                        ro_sb[:, :, :gw],
                        ro_d.ap()[:, vbase:vbase + gw].rearrange(
                            "(c p) v -> p c v", p=128))
                    nblk = (gw + 127) // 128
                    for pb in range((nblk + 1) // 2):
                        lo = lop.tile([128, 2, TH], BF16, tag="lo",
                                      name=_nm("lo_"))
                        for half in range(2):
                            bi = pb * 2 + half
                            if bi >= nblk:
                                break
                            m = min(128, gw - bi * 128)
                            for c2 in range(2):
                                pl = psm.tile([128, TQ], F32, tag="mm",
                                              name=_nm("pl_"))
                                for dc in range(ND):
                                    nc.tensor.matmul(
                                        pl[:m, :],
                                        ro_sb[:, dc,
                                              bi * 128:bi * 128 + m],
                                        vTb[:, dc,
                                            t0 + c2 * TQ:t0 + (c2 + 1) * TQ],
                                        start=(dc == 0), stop=(dc == ND - 1))
                                if c2 == 0:
                                    nc.vector.tensor_copy(
                                        lo[:m, half, 0:TQ], pl[:m, :])
                                else:
                                    nc.scalar.copy(
                                        lo[:m, half, TQ:TH], pl[:m, :])
                        vrow = vbase + pb * 256
                        nrows = min(256, gw - pb * 256)
                        if nrows > 128:
                            dst = out_d.ap()[vrow:vrow + nrows, t0:t0 + TH]
                            nc.sync.dma_start(
                                dst.rearrange("(c p) t -> p c t", p=128),
                                lo[:, 0:(nrows + 127) // 128, :])
                        else:
                            nc.sync.dma_start(
                                out_d.ap()[vrow:vrow + nrows, t0:t0 + TH],
                                lo[:nrows, 0, :])

            # ============ embedding gather + LN -> v0 -> vT ============
            idx = pp.tile([128, NT], I32)
            nc.sync.dma_start(idx[:], tok_d.ap().rearrange("(n p) -> p n", p=128))
            for n in range(NT):
                gat = gatp.tile([128, D], F32, tag="gat", name=_nm("g_"))
                nc.gpsimd.indirect_dma_start(
                    out=gat[:], out_offset=None, in_=emb_d.ap(),
                    in_offset=bass.IndirectOffsetOnAxis(ap=idx[:, n:n + 1],
                                                        axis=0),
                )
                stats = gatp.tile([128, 6], F32, tag="gs", name=_nm("gs_"))
                nc.vector.bn_stats(out=stats[:], in_=gat[:])
                mv = gatp.tile([128, 2], F32, tag="gm", name=_nm("gm_"))
                nc.vector.bn_aggr(out=mv[:], in_=stats[:])
                std = gatp.tile([128, 1], F32, tag="gd", name=_nm("gd_"))
                nc.scalar.activation(std[:], mv[:, 1:2], AF.Sqrt, bias=eps_p[:])
                rstd = gatp.tile([128, 1], F32, tag="gr", name=_nm("gr_"))
                nc.vector.reciprocal(rstd[:], std[:])
                v0 = ytp.tile([128, D], F32R, tag="yt", name=_nm("gv_"))
                nc.vector.tensor_scalar(
                    out=v0[:], in0=gat[:], scalar1=mv[:, 0:1], scalar2=rstd[:],
                    op0=OP.subtract, op1=OP.mult)
                for dc in range(ND):
                    tp = psm.tile([128, 128], F32R, tag="mm", name=_nm("et_"))
                    nc.tensor.transpose(out=tp[:],
                                        in_=v0[:, dc * 128:(dc + 1) * 128],
                                        identity=ident_r[:])
                    nc.vector.tensor_copy(vT[:, dc, n * 128:(n + 1) * 128],
                                          tp[:])
            for th in range(2):
                phaseA(th, -1)       # writes vp_td[0]
                phaseBx(th)
                phaseRope(th)

            # ================================ layers ================================
            for layer in range(N_LAYERS):
                last = layer == N_LAYERS - 1
                lnA = phaseC(0, layer)
                phaseD(0, lnA)
                lnA = phaseC(1, layer)
                phaseD(1, lnA)
                fire_ar(0)
                lnA = phaseC(2, layer)
                phaseD(2, lnA)
                # E(th0) consumes AR0 (landed during C2); next-layer A/Bx(th0)
                # hide under C3/D3 PE work (vp_td double-buffered).
                phaseE(0)
                if not last:
                    phaseA(0, layer)
                    phaseBx(0)
                else:
                    cast_vTb(0)
                lnA = phaseC(3, layer)
                phaseD(3, lnA)
                fire_ar(1)
                if not last:
                    phaseRope(0)        # DVE fills the AR1 wait
                    phaseE(1)
                    phaseA(1, layer)
                    phaseBx(1)
                    phaseRope(1)
                else:
                    if DO_READOUT:
                        readout_half(0)   # PE fills the AR1 wait
                    phaseE(1)
                    cast_vTb(1)
                    if DO_READOUT:
                        readout_half(1)

    nc.compile()
    return nc


_NC_CACHE = None


def _get_nc():
    global _NC_CACHE
    if _NC_CACHE is None:
        nc = bacc.Bacc("TRN2", target_bir_lowering=False, debug=False,
                       num_devices=8)
        _NC_CACHE = build(nc)
    return _NC_CACHE


def _rope_tables():
    import ml_dtypes
    inv_freq = (1.0 / (10000.0 ** (np.arange(0, K, 2, dtype=np.float32)
                                   / np.float32(K)))).astype(np.float32)
    t = np.arange(T, dtype=np.float32)
    freqs = (t[:, None] * inv_freq[None, :]).astype(np.float32)
    cos = np.cos(freqs).astype(np.float32)
    sin = np.sin(freqs).astype(np.float32)
    cosT = np.ascontiguousarray(cos.T).reshape(4, 128, T)
    sinT = np.ascontiguousarray(sin.T).reshape(4, 128, T)
    bf = ml_dtypes.bfloat16
    return cosT.astype(bf), sinT.astype(bf)


def kernel(input_, emb, pos, Dx, Dy, E, readout):
    import ml_dtypes
    bf = ml_dtypes.bfloat16
    input_ = np.asarray(input_)
    emb = np.ascontiguousarray(np.asarray(emb, dtype=np.float32))
    pos = np.asarray(pos, dtype=np.float32)
    Dx = np.asarray(Dx, dtype=np.float32)
    Dy = np.asarray(Dy, dtype=np.float32)
    E = np.asarray(E, dtype=np.float32)
    readout = np.asarray(readout, dtype=np.float32)

    nc = _get_nc()
    cosT, sinT = _rope_tables()
    posT = np.ascontiguousarray(pos.T).astype(bf)
    ps = pos.sum(axis=1) / np.float32(D)
    nmw = np.ascontiguousarray(
        np.broadcast_to((-ps)[None, :], (128, T)).astype(bf))
    mw2 = np.ascontiguousarray((ps * ps)[None, :].astype(bf))

    in_maps = []
    for c in range(8):
        b, h = divmod(c, 4)
        in_maps.append({
            "tok": np.ascontiguousarray(input_[b].astype(np.int32)),
            "emb": emb,
            "posT": posT,
            "dx": np.ascontiguousarray(Dx[h]).astype(bf),
            "dy": np.ascontiguousarray(Dy[h]).astype(bf),
            "eh": np.ascontiguousarray(E[h * K:(h + 1) * K]).astype(bf),
            "ro": np.ascontiguousarray(readout[:, h * VS:(h + 1) * VS]).astype(bf),
            "cosh": cosT,
            "sinh": sinT,
            "nmw": nmw,
            "mw2": mw2,
        })
    trace = os.environ.get("KRN_TRACE", "0") == "1"
    res = run_bass_kernel_spmd(nc, in_maps, list(range(8)), trace=trace)
    out = np.empty((B, T, V), dtype=np.float32)
    for c in range(8):
        b, h = divmod(c, 4)
        out[b, :, h * VS:(h + 1) * VS] = res.results[c]["logitsT"].astype(
            np.float32).T
    kernel._last_results = res
    return out


# revision 34
# speedup vs baseline: 1.1002x; 1.1002x over previous
"""Trainium2 Bass kernel for nn_BDH_4406636445711 (dense transformer).

Sharding: 8 cores = data-parallel over B(2) x tensor-parallel over H(4).
Core c handles (b = c//4, h = c%4): its head's Dx/Dy slices, E rows, and a
V/4 shard of the readout. Per layer the y@E partial is AllReduced within
each b-group of 4 cores; v stays replicated inside the group. The host
stitches the 8 per-core [VS, T] logit shards into the full [B, T, V].

v4 design notes:
- Quarter-T (512) granularity C->ln->D pipeline; PSUM = pmm512(2) +
  psacc(4) + pstat(2) = 8 banks.
- bf16 everywhere except the f32r residual vT, LN stat chains, and pos.
- x lives in SBUF (xsb, 32KB/part) - no DRAM spill.
- ln chains: colsum matmuls -> ACT mul/Square -> DVE var ->
  ACT Sqrt -> DVE reciprocal_approx_fast -> GPSIMD partition_broadcast.
- ln(w) mean shortcut: sum_d w = sum_d pos (v, ln(u) are LN outputs).
- Aggressive overlap: E(th0)/phaseA(th0)/B_x(th0) emitted mid-layer
  (after D(q2)) under C-phase PE work; vp_td double-buffered per layer so
  next-layer transposes don't WAR current a-accum; RoPE(th0) overlaps
  E(th1)'s AllReduce wait; next-layer C(q0) kc-loop walks RoPE pair order
  so scores chase RoPE output. Last layer: readout(th0) fills AR(th1).
"""

import os
import sys

sys.path.insert(0, "/opt/trn_rl_repo")

import numpy as np

import concourse.bass as bass
import concourse.tile as tile
from concourse import bacc, mybir
from concourse.bass_utils import run_bass_kernel_spmd
from concourse.masks import make_identity
from concourse import library_config

F32 = mybir.dt.float32
F32R = mybir.dt.float32r
BF16 = mybir.dt.bfloat16
I32 = mybir.dt.int32
AF = mybir.ActivationFunctionType
OP = mybir.AluOpType

B, T, H, D, K, V, L = 2, 2048, 4, 256, 1024, 32000, 6
VS = V // 4
EPS = 1e-5
NT = T // 128
NKT = K // 128
ND = D // 128
TH = T // 2
TQ = 512
NQ = T // TQ
PAIR_KC = [0, 4, 1, 5, 2, 6, 3, 7]   # kc order matching RoPE pair emission

N_LAYERS = int(os.environ.get("KRN_LAYERS", str(L)))
DO_READOUT = os.environ.get("KRN_READOUT", "1") == "1"


def build(nc):
    tok_d = nc.dram_tensor("tok", [T], I32, kind="ExternalInput")
    emb_d = nc.dram_tensor("emb", [V, D], F32, kind="ExternalInput")
    posT_d = nc.dram_tensor("posT", [D, T], BF16, kind="ExternalInput")
    dx_d = nc.dram_tensor("dx", [D, K], BF16, kind="ExternalInput")
    dy_d = nc.dram_tensor("dy", [D, K], BF16, kind="ExternalInput")
    e_d = nc.dram_tensor("eh", [K, D], BF16, kind="ExternalInput")
    ro_d = nc.dram_tensor("ro", [D, VS], BF16, kind="ExternalInput")
    cos_d = nc.dram_tensor("cosh", [4, 128, T], BF16, kind="ExternalInput")
    sin_d = nc.dram_tensor("sinh", [4, 128, T], BF16, kind="ExternalInput")
    nmw_d = nc.dram_tensor("nmw", [128, T], BF16, kind="ExternalInput")
    mw2_d = nc.dram_tensor("mw2", [1, T], BF16, kind="ExternalInput")
    out_d = nc.dram_tensor("logitsT", [VS, T], BF16, kind="ExternalOutput")

    groups = [[0, 1, 2, 3], [4, 5, 6, 7]]

    from contextlib import ExitStack

    with tile.TileContext(nc) as tc, ExitStack() as es:
        ent = es.enter_context
        ent(nc.allow_low_precision(reason="bf16/f32r rounding is intentional"))
        pp = ent(tc.tile_pool(name="persist", bufs=1))
        csp = ent(tc.tile_pool(name="cs", bufs=1))       # [128,8,TH] bf16 cos+sin
        rpp = ent(tc.tile_pool(name="rope", bufs=2))     # [128,TH] bf16
        scp = ent(tc.tile_pool(name="scr", bufs=2))      # [128,TQ] bf16
        atp = ent(tc.tile_pool(name="at", bufs=1))       # [128,ND,TQ] f32r
        sqp = ent(tc.tile_pool(name="sq", bufs=1))       # [128,ND,TQ] f32r
        lnp = ent(tc.tile_pool(name="ln", bufs=2))       # [128,ND,TQ] bf16
        ytp = ent(tc.tile_pool(name="yt", bufs=2))       # [128,TQ] bf16
        yep = ent(tc.tile_pool(name="ye", bufs=1))       # [128,ND,TQ] bf16
        utp = ent(tc.tile_pool(name="ut", bufs=1))       # [128,ND,TH] bf16
        squp = ent(tc.tile_pool(name="squ", bufs=1))     # [128,ND,TQ] f32r
        smp = ent(tc.tile_pool(name="sm", bufs=4))       # [1,TQ] f32 smalls
        bcp = ent(tc.tile_pool(name="bc", bufs=2))       # [128,TH] f32 bcasts
        tmpp = ent(tc.tile_pool(name="tmp", bufs=2))     # [128,TH] f32 temps
        posp = ent(tc.tile_pool(name="pos", bufs=1))     # [128,ND,TH] bf16
        gatp = ent(tc.tile_pool(name="gat", bufs=2))     # embed tiles
        lop = ent(tc.tile_pool(name="lo", bufs=1))       # [128,2,TH] bf16
        rop = ent(tc.tile_pool(name="rob", bufs=1))      # [128,ND,1024] bf16
        psm = ent(tc.tile_pool(name="pmm512", bufs=2, space="PSUM"))
        psa = ent(tc.tile_pool(name="psacc", bufs=2, space="PSUM"))
        pst = ent(tc.tile_pool(name="pstat", bufs=2, space="PSUM"))
        dpool = ent(tc.tile_pool(name="dram", bufs=1, space="DRAM"))
        if True:
            _ctr = [0]

            def _nm(p):
                _ctr[0] += 1
                return f"{p}{_ctr[0]}"

            # ---- constants ----
            ident_f = smp.tile([128, 128], F32, tag="sm", name="identf")
            make_identity(nc, ident_f[:])
            ident_r = pp.tile([128, 128], F32R)
            nc.vector.tensor_copy(ident_r[:], ident_f[:])
            ones_pf = pp.tile([128, 1], F32)
            nc.vector.memset(ones_pf[:], 1.0)
            ones_p = pp.tile([128, 1], F32R)
            nc.vector.tensor_copy(ones_p[:], ones_pf[:])
            ones_b = pp.tile([128, 1], BF16)
            nc.vector.tensor_copy(ones_b[:], ones_pf[:])
            eps_p = pp.tile([128, 1], F32)
            nc.vector.memset(eps_p[:], EPS)
            eps_1 = pp.tile([1, 1], F32)
            nc.vector.memset(eps_1[:], EPS)
            nc.gpsimd.load_library(library_config.attn)

            # ---- persistent tensors ----
            vT = pp.tile([128, ND, T], F32R)       # residual master (dT layout)
            vTb = pp.tile([128, ND, T], BF16)      # bf16 shadow
            qT = pp.tile([128, NKT, T], BF16)      # RoPE'd q (kT layout)
            xsb = pp.tile([128, NKT, T], BF16)     # x = relu(v@Dx) (kT layout)
            vp_td = [pp.tile([128, NT, D], BF16, name=f"vp{i}") for i in range(2)]
            dx_sb = pp.tile([128, ND, K], BF16)
            nc.sync.dma_start(dx_sb[:], dx_d.ap().rearrange("(c p) k -> p c k", p=128))
            dy_sb = pp.tile([128, ND, K], BF16)
            nc.sync.dma_start(dy_sb[:], dy_d.ap().rearrange("(c p) k -> p c k", p=128))
            e_sb = pp.tile([128, NKT, D], BF16)
            nc.sync.dma_start(e_sb[:], e_d.ap().rearrange("(c p) d -> p c d", p=128))
            nmw_sb = pp.tile([128, T], BF16)
            nc.sync.dma_start(nmw_sb[:], nmw_d.ap())
            mw2_sb = pp.tile([1, T], BF16)
            nc.sync.dma_start(mw2_sb[:], mw2_d.ap())

            cc_in = [dpool.tile([ND, 128, TH], BF16, tag=f"cci{i}", name=f"cci{i}")
                     for i in range(2)]
            cc_out = [dpool.tile([ND, 128, TH], BF16, tag=f"cco{i}", name=f"cco{i}")
                      for i in range(2)]

            def ln_chain(s1, s2, want_nm, mw2_ap, nm_dst, rs_dst):
                """LN stats from [1,TQ] psum colsums into bcast tile slices."""
                if want_nm:
                    nm_sb = smp.tile([1, TQ], F32, tag="sm", name=_nm("nm_"))
                    nc.scalar.mul(nm_sb[:], s1, -1.0 / D)
                    m2 = smp.tile([1, TQ], F32, tag="sm", name=_nm("m2_"))
                    nc.scalar.activation(m2[:], nm_sb[:], AF.Square)
                    nc.gpsimd.partition_broadcast(nm_dst, nm_sb[:])
                    m2_ap = m2[:]
                else:
                    m2_ap = mw2_ap
                var = smp.tile([1, TQ], F32, tag="sm", name=_nm("var_"))
                nc.vector.scalar_tensor_tensor(
                    out=var[:], in0=s2, scalar=1.0 / D, in1=m2_ap,
                    op0=OP.mult, op1=OP.subtract)
                std = smp.tile([1, TQ], F32, tag="sm", name=_nm("std_"))
                nc.scalar.activation(std[:], var[:], AF.Sqrt, bias=eps_1[:])
                rs = smp.tile([1, TQ], F32, tag="sm", name=_nm("rs_"))
                nc.vector.reciprocal_approx_fast(rs[:], std[:])
                nc.gpsimd.partition_broadcast(rs_dst, rs[:])

            def colsums(dst, src_bf=None, src_f=None):
                """dst [1,TQ] psum += ones^T @ src over ND blocks."""
                for dc in range(ND):
                    if src_bf is not None:
                        nc.tensor.matmul(dst, ones_b[:], src_bf(dc),
                                         start=(dc == 0), stop=(dc == ND - 1),
                                         skip_group_check=True)
                    else:
                        nc.tensor.matmul(dst, ones_p[:], src_f(dc),
                                         start=(dc == 0), stop=(dc == ND - 1),
                                         skip_group_check=True)

            def phaseA(th, layer):
                """vT[:, th] += pos; vTb cast; transpose into vp_td[next]."""
                t0 = th * TH
                vp = vp_td[(layer + 1) % 2]
                for dc in range(ND):
                    pch = posp.tile([128, TH], BF16, tag="pos", name=_nm("pos_"))
                    nc.sync.dma_start(
                        pch[:], posT_d.ap()[dc * 128:(dc + 1) * 128,
                                            t0:t0 + TH])
                    nc.vector.tensor_add(
                        vT[:, dc, t0:t0 + TH],
                        vT[:, dc, t0:t0 + TH].bitcast(F32), pch[:])
                    nc.vector.tensor_copy(vTb[:, dc, t0:t0 + TH],
                                          vT[:, dc, t0:t0 + TH])
                for dc in range(ND):
                    for g in range(2):
                        tp = psm.tile([128, 4, 128], F32R, tag="mm", name=_nm("tp_"))
                        for k in range(4):
                            n = th * 8 + g * 4 + k
                            nc.tensor.transpose(
                                out=tp[:, k, :],
                                in_=vT[:, dc, n * 128:(n + 1) * 128],
                                identity=ident_r[:])
                        nc.vector.tensor_copy(
                            vp[:, th * 8 + g * 4:th * 8 + (g + 1) * 4,
                               dc * 128:(dc + 1) * 128], tp[:])

            def phaseBx(th):
                """x = relu(v @ Dx) for this t-half into xsb (ACT writes)."""
                t0 = th * TH
                for ii in range(NKT):
                    for qq in range(2):
                        px = psm.tile([128, TQ], F32, tag="mm", name=_nm("px_"))
                        for dc in range(ND):
                            nc.tensor.matmul(
                                px[:],
                                dx_sb[:, dc, ii * 128:(ii + 1) * 128],
                                vTb[:, dc, t0 + qq * TQ:t0 + (qq + 1) * TQ],
                                start=(dc == 0), stop=(dc == ND - 1))
                        nc.scalar.activation(
                            xsb[:, ii, t0 + qq * TQ:t0 + (qq + 1) * TQ],
                            px[:], AF.Relu)

            def phaseRope(th):
                """qT = RoPE(x) for this t-half (DVE, bf16, pair order)."""
                t0 = th * TH
                gsl = slice(t0, t0 + TH)
                for i in range(4):
                    if i % 2 == 0:
                        cs = csp.tile([128, 4, TH], BF16, tag="cs",
                                      name=_nm("cs_"))
                        nc.sync.dma_start(
                            cs[:, 0:2, :],
                            cos_d.ap()[i:i + 2, :, t0:t0 + TH].rearrange(
                                "c p t -> p c t"))
                        nc.sync.dma_start(
                            cs[:, 2:4, :],
                            sin_d.ap()[i:i + 2, :, t0:t0 + TH].rearrange(
                                "c p t -> p c t"))
                    xi = xsb[:, i, gsl]
                    xj = xsb[:, i + 4, gsl]
                    cos_t = cs[:, i % 2, :]
                    sin_t = cs[:, 2 + i % 2, :]
                    m1 = rpp.tile([128, TH], BF16, tag="rp", name=_nm("m1_"))
                    nc.vector.tensor_mul(m1[:], xi, cos_t)
                    m2 = rpp.tile([128, TH], BF16, tag="rp", name=_nm("m2_"))
                    nc.vector.tensor_mul(m2[:], xj, sin_t)
                    nc.vector.tensor_sub(qT[:, i, gsl], m1[:], m2[:])
                    m3 = rpp.tile([128, TH], BF16, tag="rp", name=_nm("m3_"))
                    nc.vector.tensor_mul(m3[:], xj, cos_t)
                    m4 = rpp.tile([128, TH], BF16, tag="rp", name=_nm("m4_"))
                    nc.vector.tensor_mul(m4[:], xi, sin_t)
                    nc.vector.tensor_add(qT[:, i + 4, gsl], m3[:], m4[:])

            def phaseC(q, layer):
                """scores for quarter q; a-accum; ln(a) -> lnA (bf16)."""
                q0 = q * TQ
                vp = vp_td[layer % 2]
                pa = psa.tile([128, ND, TQ], F32, tag="acc", name=_nm("pa_"))
                for j in range(NT):
                    pscr = psm.tile([128, TQ], F32, tag="mm", name=_nm("ps_"))
                    for ki, kc in enumerate(PAIR_KC):
                        nc.tensor.matmul(
                            pscr[:],
                            qT[:, kc, j * 128:(j + 1) * 128],
                            qT[:, kc, q0:q0 + TQ],
                            start=(ki == 0), stop=(ki == NKT - 1))
                    scr = scp.tile([128, TQ], BF16, tag="scr", name=_nm("sc_"))
                    nc.scalar.copy(scr[:], pscr[:])
                    for dc in range(ND):
                        nc.tensor.matmul(
                            pa[:, dc, :],
                            vp[:, j, dc * 128:(dc + 1) * 128],
                            scr[:],
                            start=(j == 0), stop=(j == NT - 1),
                            skip_group_check=True)
                aT = atp.tile([128, ND, TQ], BF16, tag="at", name=_nm("aT_"))
                nc.vector.tensor_copy(aT[:], pa[:])
                sq = sqp.tile([128, ND, TQ], BF16, tag="sq", name=_nm("sq_"))
                nc.scalar.activation(sq[:], pa[:].bitcast(F32), AF.Square)
                s1t = pst.tile([1, TQ], F32, tag="st", name=_nm("s1_"))
                colsums(s1t[:], src_bf=lambda dc: aT[:, dc, :])
                s2t = pst.tile([1, TQ], F32, tag="st", name=_nm("s2_"))
                colsums(s2t[:], src_bf=lambda dc: sq[:, dc, :])
                nm_b = bcp.tile([128, TQ], F32, tag="bc", name=_nm("nmb_"))
                rs_b = bcp.tile([128, TQ], F32, tag="bc", name=_nm("rsb_"))
                ln_chain(s1t[:], s2t[:], True, None, nm_b[:], rs_b[:])
                lnA = lnp.tile([128, ND, TQ], BF16, tag="ln", name=_nm("lnA_"))
                for dc in range(ND):
                    cent = ytp.tile([128, TQ], BF16, tag="yt", name=_nm("ce_"))
                    nc.vector.tensor_add(cent[:], aT[:, dc, :], nm_b[:])
                    nc.vector.tensor_mul(lnA[:, dc, :], cent[:], rs_b[:])
                return lnA

            def phaseD(q, lnA):
                """y_i = relu(lnA @ Dy_i) * x_i; yE accum; ship to cc_in."""
                q0 = q * TQ
                pye = psa.tile([128, ND, TQ], F32, tag="acc", name=_nm("pye_"))
                for i in range(NKT):
                    py = psm.tile([128, TQ], F32, tag="mm", name=_nm("py_"))
                    for dc in range(ND):
                        nc.tensor.matmul(
                            py[:],
                            dy_sb[:, dc, i * 128:(i + 1) * 128],
                            lnA[:, dc, :],
                            start=(dc == 0), stop=(dc == ND - 1))
                    yt = ytp.tile([128, TQ], BF16, tag="yt", name=_nm("yt_"))
                    nc.vector.scalar_tensor_tensor(
                        out=yt[:], in0=py[:], scalar=0.0,
                        in1=xsb[:, i, q0:q0 + TQ],
                        op0=OP.max, op1=OP.mult)
                    for dc in range(ND):
                        nc.tensor.matmul(
                            pye[:, dc, :],
                            e_sb[:, i, dc * 128:(dc + 1) * 128],
                            yt[:],
                            start=(i == 0), stop=(i == NKT - 1),
                            skip_group_check=True)
                ye = yep.tile([128, ND, TQ], BF16, tag="ye", name=_nm("ye_"))
                nc.vector.tensor_copy(ye[:], pye[:])
                th, qq = q // 2, q % 2
                nc.sync.dma_start(
                    cc_in[th][:][:, :, qq * TQ:(qq + 1) * TQ].rearrange(
                        "c p t -> p c t"), ye[:])

            def phaseE(th):
                """u = ln(AR sum); w = vp + u; v = ln(w)."""
                t0 = th * TH
                uT = utp.tile([128, ND, TH], BF16, tag="ut", name=_nm("uT_"))
                nc.sync.dma_start(
                    uT[:], cc_out[th][:].rearrange("c p t -> p c t"))
                nmu_b = bcp.tile([128, TH], F32, tag="bcth", name=_nm("nmu_"))
                rsu_b = bcp.tile([128, TH], F32, tag="bcth", name=_nm("rsu_"))
                rsw_b = bcp.tile([128, TH], F32, tag="bcth", name=_nm("rsw_"))
                for qq in range(2):
                    sl = slice(qq * TQ, (qq + 1) * TQ)
                    squ = squp.tile([128, ND, TQ], BF16, tag="squ", name=_nm("su_"))
                    nc.scalar.activation(squ[:], uT[:, :, sl], AF.Square)
                    s1u = pst.tile([1, TQ], F32, tag="st", name=_nm("s1u_"))
                    colsums(s1u[:], src_bf=lambda dc: uT[:, dc, sl])
                    s2u = pst.tile([1, TQ], F32, tag="st", name=_nm("s2u_"))
                    colsums(s2u[:], src_bf=lambda dc: squ[:, dc, :])
                    ln_chain(s1u[:], s2u[:], True, None,
                             nmu_b[:, sl], rsu_b[:, sl])
                for dc in range(ND):
                    gsl = slice(t0, t0 + TH)
                    cent = tmpp.tile([128, TH], F32, tag="tmpth", name=_nm("cu_"))
                    nc.vector.tensor_add(cent[:], uT[:, dc, :], nmu_b[:])
                    lnu = tmpp.tile([128, TH], F32, tag="tmpth", name=_nm("lu_"))
                    nc.vector.tensor_mul(lnu[:], cent[:], rsu_b[:])
                    nc.vector.tensor_add(vT[:, dc, gsl],
                                         vT[:, dc, gsl].bitcast(F32), lnu[:])
                # ln(w): mean precomputed from pos colsums
                for qq in range(2):
                    sl = slice(qq * TQ, (qq + 1) * TQ)
                    gsl = slice(t0 + qq * TQ, t0 + (qq + 1) * TQ)
                    sqw = squp.tile([128, ND, TQ], BF16, tag="squ", name=_nm("sw_"))
                    nc.scalar.activation(sqw[:], vT[:, :, gsl].bitcast(F32),
                                         AF.Square)
                    s2w = pst.tile([1, TQ], F32, tag="st", name=_nm("s2w_"))
                    colsums(s2w[:], src_bf=lambda dc: sqw[:, dc, :])
                    ln_chain(None, s2w[:], False,
                             mw2_sb[:, t0 + qq * TQ:t0 + (qq + 1) * TQ],
                             None, rsw_b[:, sl])
                for dc in range(ND):
                    gsl = slice(t0, t0 + TH)
                    cent = tmpp.tile([128, TH], F32, tag="tmpth", name=_nm("cw_"))
                    nc.vector.tensor_add(cent[:], vT[:, dc, gsl].bitcast(F32),
                                         nmw_sb[:, gsl])
                    nc.vector.tensor_mul(vT[:, dc, gsl], cent[:], rsw_b[:])

            def fire_ar(th):
                nc.gpsimd.collective_compute(
                    "AllReduce", OP.add, replica_groups=groups,
                    ins=[cc_in[th][:].opt()], outs=[cc_out[th][:].opt()])

            def cast_vTb(th):
                t0 = th * TH
                for dc in range(ND):
                    nc.vector.tensor_copy(vTb[:, dc, t0:t0 + TH],
                                          vT[:, dc, t0:t0 + TH])

            def readout_half(th):
                t0 = th * TH
                nvg = 8  # groups of 8 vocab blocks (last ragged)
                for vg in range(nvg):
                    vbase = vg * 1024
                    gw = min(1024, VS - vbase)
                    ro_sb = rop.tile([128, ND, 1024], BF16, tag="ro",
                                     name=_nm("ro_"))
                    nc.sync.dma_start(
                        ro_sb[:, :, :gw],
                        ro_d.ap()[:, vbase:vbase + gw].rearrange(
                            "(c p) v -> p c v", p=128))
                    nblk = (gw + 127) // 128
                    for pb in range((nblk + 1) // 2):
                        lo = lop.tile([128, 2, TH], BF16, tag="lo",
                                      name=_nm("lo_"))
                        for half in range(2):
                            bi = pb * 2 + half
                            if bi >= nblk:
                                break
                            m = min(128, gw - bi * 128)
                            for c2 in range(2):
                                pl = psm.tile([128, TQ], F32, tag="mm",
                                              name=_nm("pl_"))
                                for dc in range(ND):
                                    nc.tensor.matmul(
                                        pl[:m, :],
                                        ro_sb[:, dc,
                                              bi * 128:bi * 128 + m],
                                        vTb[:, dc,
                                            t0 + c2 * TQ:t0 + (c2 + 1) * TQ],
                                        start=(dc == 0), stop=(dc == ND - 1))
                                if c2 == 0:
                                    nc.vector.tensor_copy(
                                        lo[:m, half, 0:TQ], pl[:m, :])
                                else:
                                    nc.scalar.copy(
                                        lo[:m, half, TQ:TH], pl[:m, :])
                        vrow = vbase + pb * 256
                        nrows = min(256, gw - pb * 256)
                        if nrows > 128:
                            dst = out_d.ap()[vrow:vrow + nrows, t0:t0 + TH]
                            nc.sync.dma_start(
                                dst.rearrange("(c p) t -> p c t", p=128),
                                lo[:, 0:(nrows + 127) // 128, :])
                        else:
                            nc.sync.dma_start(
                                out_d.ap()[vrow:vrow + nrows, t0:t0 + TH],
                                lo[:nrows, 0, :])

            # ============ embedding gather + LN -> v0 -> vT ============
            idx = pp.tile([128, NT], I32)
            nc.sync.dma_start(idx[:], tok_d.ap().rearrange("(n p) -> p n", p=128))
            for n in range(NT):
                gat = gatp.tile([128, D], F32, tag="gat", name=_nm("g_"))
                nc.gpsimd.indirect_dma_start(
                    out=gat[:], out_offset=None, in_=emb_d.ap(),
                    in_offset=bass.IndirectOffsetOnAxis(ap=idx[:, n:n + 1],
                                                        axis=0),
                )
                stats = gatp.tile([128, 6], F32, tag="gs", name=_nm("gs_"))
                nc.vector.bn_stats(out=stats[:], in_=gat[:])
                mv = gatp.tile([128, 2], F32, tag="gm", name=_nm("gm_"))
                nc.vector.bn_aggr(out=mv[:], in_=stats[:])
                std = gatp.tile([128, 1], F32, tag="gd", name=_nm("gd_"))
                nc.scalar.activation(std[:], mv[:, 1:2], AF.Sqrt, bias=eps_p[:])
                rstd = gatp.tile([128, 1], F32, tag="gr", name=_nm("gr_"))
                nc.vector.reciprocal(rstd[:], std[:])
                v0 = ytp.tile([128, D], F32R, tag="yt", name=_nm("gv_"))
                nc.vector.tensor_scalar(
                    out=v0[:], in0=gat[:], scalar1=mv[:, 0:1], scalar2=rstd[:],
                    op0=OP.subtract, op1=OP.mult)
                for dc in range(ND):
                    tp = psm.tile([128, 128], F32R, tag="mm", name=_nm("et_"))
                    nc.tensor.transpose(out=tp[:],
                                        in_=v0[:, dc * 128:(dc + 1) * 128],
                                        identity=ident_r[:])
                    nc.vector.tensor_copy(vT[:, dc, n * 128:(n + 1) * 128],
                                          tp[:])
            for th in range(2):
                phaseA(th, -1)       # writes vp_td[0]
                phaseBx(th)
                phaseRope(th)

            # ================================ layers ================================
            for layer in range(N_LAYERS):
                last = layer == N_LAYERS - 1
                # Software-pipelined quarters: C(q+1) is emitted before
                # D(q) so each ln(a) stats chain hides under the next
                # quarter's scores instead of stalling the in-order PE queue.
                lnA0 = phaseC(0, layer)
                lnA1 = phaseC(1, layer)
                phaseD(0, lnA0)
                lnA2 = phaseC(2, layer)
                phaseD(1, lnA1)
                fire_ar(0)
                lnA3 = phaseC(3, layer)
                phaseD(2, lnA2)
                # E(th0) consumes AR0 (landed during C3); next-layer A/Bx(th0)
                # hide under D2/D3 PE work (vp_td double-buffered).
                phaseE(0)
                if not last:
                    phaseA(0, layer)
                    phaseBx(0)
                else:
                    cast_vTb(0)
                phaseD(3, lnA3)
                fire_ar(1)
                if not last:
                    phaseRope(0)        # DVE fills the AR1 wait
                    phaseE(1)
                    phaseA(1, layer)
                    phaseBx(1)
                    phaseRope(1)
                else:
                    if DO_READOUT:
                        readout_half(0)   # PE fills the AR1 wait
                    phaseE(1)
                    cast_vTb(1)
                    if DO_READOUT:
                        readout_half(1)

    nc.compile()
    return nc


_NC_CACHE = None


def _get_nc():
    global _NC_CACHE
    if _NC_CACHE is None:
        nc = bacc.Bacc("TRN2", target_bir_lowering=False, debug=False,
                       num_devices=8)
        _NC_CACHE = build(nc)
    return _NC_CACHE


def _rope_tables():
    import ml_dtypes
    inv_freq = (1.0 / (10000.0 ** (np.arange(0, K, 2, dtype=np.float32)
                                   / np.float32(K)))).astype(np.float32)
    t = np.arange(T, dtype=np.float32)
    freqs = (t[:, None] * inv_freq[None, :]).astype(np.float32)
    cos = np.cos(freqs).astype(np.float32)
    sin = np.sin(freqs).astype(np.float32)
    cosT = np.ascontiguousarray(cos.T).reshape(4, 128, T)
    sinT = np.ascontiguousarray(sin.T).reshape(4, 128, T)
    bf = ml_dtypes.bfloat16
    return cosT.astype(bf), sinT.astype(bf)


def kernel(input_, emb, pos, Dx, Dy, E, readout):
    import ml_dtypes
    bf = ml_dtypes.bfloat16
    input_ = np.asarray(input_)
    emb = np.ascontiguousarray(np.asarray(emb, dtype=np.float32))
    pos = np.asarray(pos, dtype=np.float32)
    Dx = np.asarray(Dx, dtype=np.float32)
    Dy = np.asarray(Dy, dtype=np.float32)
    E = np.asarray(E, dtype=np.float32)
    readout = np.asarray(readout, dtype=np.float32)

    nc = _get_nc()
    cosT, sinT = _rope_tables()
    posT = np.ascontiguousarray(pos.T).astype(bf)
    ps = pos.sum(axis=1) / np.float32(D)
    nmw = np.ascontiguousarray(
        np.broadcast_to((-ps)[None, :], (128, T)).astype(bf))
    mw2 = np.ascontiguousarray((ps * ps)[None, :].astype(bf))

    in_maps = []
    for c in range(8):
        b, h = divmod(c, 4)
        in_maps.append({
            "tok": np.ascontiguousarray(input_[b].astype(np.int32)),
            "emb": emb,
            "posT": posT,
            "dx": np.ascontiguousarray(Dx[h]).astype(bf),
            "dy": np.ascontiguousarray(Dy[h]).astype(bf),
            "eh": np.ascontiguousarray(E[h * K:(h + 1) * K]).astype(bf),
            "ro": np.ascontiguousarray(readout[:, h * VS:(h + 1) * VS]).astype(bf),
            "cosh": cosT,
            "sinh": sinT,
            "nmw": nmw,
            "mw2": mw2,
        })
    trace = os.environ.get("KRN_TRACE", "0") == "1"
    res = run_bass_kernel_spmd(nc, in_maps, list(range(8)), trace=trace)
    out = np.empty((B, T, V), dtype=np.float32)
    for c in range(8):
        b, h = divmod(c, 4)
        out[b, :, h * VS:(h + 1) * VS] = res.results[c]["logitsT"].astype(
            np.float32).T
    kernel._last_results = res
    return out
